# revision 4
# baseline (speedup 1.0000x reference)
"""Grouped-Query Attention (B=1, L=4096, D=1024, 16 q-heads, 4 kv-heads, hd=64)
on 8 Trainium2 NeuronCores.

Sharding: core c owns q-heads {2c, 2c+1} and their shared kv-head c//2.
Each core computes Q/K/V projections for its heads from the full (replicated)
x, runs dense softmax attention for its 2 heads, and produces a partial
output projection  attn_heads @ Wo[head_rows]  of full shape [4096, 1024].
Host sums the 8 partials and adds bo (row-parallel all-reduce done on host).

On-chip dataflow (per core, everything bf16 on the PE array, fp32 in PSUM):
  x^T [1024,4096] (host-pretransposed, bf16) -> SBUF
  K^T[64,L], Q^T[64,2,L] (pre-scaled by 1/8), V[L,64|ones] via PE matmuls
  per (head, 512-wide q-block):
     S^T k-tile [128k, 512q] = (K^T k-slice).T @ Q^T_h   (PE, C=64)
     P^T = exp(S^T)                                      (ACT, grouped 3 banks)
     outT[65, 512] += (V-tile|ones).T @ P^T-tile         (PE, C=128; row 64 = denom)
     normalize: recip(denom) broadcast via rank-1 matmul, DVE multiply
  out[Lchunk,1024] = sum_h attnT_h.T @ Wo_h              (PE, C=64 x2)
"""

import os

os.environ.setdefault("MYCRO_LOCAL_CACHE", "1")

import numpy as np
import ml_dtypes

import concourse.bass as bass
import concourse.bacc as bacc
import concourse.mybir as mybir
from concourse.tile import TileContext
from concourse.bass_utils import run_bass_kernel_spmd

BF16 = mybir.dt.bfloat16
F32 = mybir.dt.float32
AF = mybir.ActivationFunctionType

D = 1024
L = 4096
NHEAD = 16
NKV = 4
HD = 64
NCORES = 8
HPC = NHEAD // NCORES  # 2 q heads per core
QB = 512               # q-block width
NQB = L // QB          # 8
KT = 128               # k-tile
NKT = L // KT          # 32
KG = 3                 # k-tiles per exp group (3 PSUM banks)
NF = D // 128          # 8 feature chunks
SCALE = 0.125          # 1/sqrt(64)

_CACHE = {}


def _build():
    nc = bacc.Bacc("TRN2", target_bir_lowering=False, debug=False)

    xT = nc.declare_dram_parameter("xT", [D, L], BF16, isOutput=False)
    wq = nc.declare_dram_parameter("wq", [D, HPC * HD], BF16, isOutput=False)
    wk = nc.declare_dram_parameter("wk", [D, HD], BF16, isOutput=False)
    wv = nc.declare_dram_parameter("wv", [D, HD], BF16, isOutput=False)
    wo0 = nc.declare_dram_parameter("wo0", [HD, D], BF16, isOutput=False)
    wo1 = nc.declare_dram_parameter("wo1", [HD, D], BF16, isOutput=False)
    bq = nc.declare_dram_parameter("bq", [1, HPC * HD], BF16, isOutput=False)
    bk = nc.declare_dram_parameter("bk", [1, HD], BF16, isOutput=False)
    bv = nc.declare_dram_parameter("bv", [1, HD], BF16, isOutput=False)
    out = nc.declare_dram_parameter("out", [L, D], F32, isOutput=True)

    with TileContext(nc) as tc:
        with (
            tc.tile_pool(name="sing", bufs=1) as sing,
            tc.tile_pool(name="ptp", bufs=3) as ptp,
            tc.tile_pool(name="attp", bufs=2) as attp,
            tc.tile_pool(name="nrm", bufs=3) as nrm,
            tc.tile_pool(name="obp", bufs=3) as obp,
            tc.tile_pool(name="psA", bufs=2, space="PSUM") as psA,
            tc.tile_pool(name="psB", bufs=1, space="PSUM") as psB,
            tc.tile_pool(name="psC", bufs=1, space="PSUM") as psC,
        ):
            # ---- resident SBUF tensors ----
            xT_sb = sing.tile([128, NF, L], BF16)
            wq_sb = sing.tile([128, NF, HPC * HD], BF16)
            wk_sb = sing.tile([128, NF, HD], BF16)
            wv_sb = sing.tile([128, NF, HD], BF16)
            wo0_sb = sing.tile([HD, D], BF16)
            wo1_sb = sing.tile([HD, D], BF16)
            bq_sb = sing.tile([1, HPC * HD], BF16)
            bk_sb = sing.tile([1, HD], BF16)
            bv_sb = sing.tile([1, HD], BF16)
            ones_b = sing.tile([1, QB], BF16)       # bf16 ones row (bias rank-1)
            ones_f = sing.tile([65, HD], F32)       # fp32 ones (recip bcast, row 64)
            KT_sb = sing.tile([HD, L], BF16)
            QT_sb = sing.tile([HD, HPC, L], BF16)
            V_sb = sing.tile([128, NKT, HD + 1], BF16)  # col 64 = 1.0 (denom)

            for f in range(NF):
                fs = slice(128 * f, 128 * (f + 1))
                nc.sync.dma_start(out=xT_sb[:, f, :], in_=xT[fs, :])
                nc.sync.dma_start(out=wq_sb[:, f, :], in_=wq[fs, :])
                nc.sync.dma_start(out=wk_sb[:, f, :], in_=wk[fs, :])
                nc.sync.dma_start(out=wv_sb[:, f, :], in_=wv[fs, :])
            nc.sync.dma_start(out=wo0_sb, in_=wo0[:, :])
            nc.sync.dma_start(out=wo1_sb, in_=wo1[:, :])
            nc.sync.dma_start(out=bq_sb, in_=bq[:, :])
            nc.sync.dma_start(out=bk_sb, in_=bk[:, :])
            nc.sync.dma_start(out=bv_sb, in_=bv[:, :])
            nc.gpsimd.memset(ones_b, 1.0)
            nc.gpsimd.memset(ones_f, 1.0)
            nc.gpsimd.memset(V_sb[:, :, HD], 1.0)

            # ---- projections ----
            # K^T[64, L] = Wk.T @ x^T  (+ bk)
            for n in range(NQB):
                ns = slice(QB * n, QB * (n + 1))
                kps = psA.tile([HD, QB], F32, tag="st")
                for f in range(NF):
                    nc.tensor.matmul(kps, wk_sb[:, f, :], xT_sb[:, f, ns],
                                     start=(f == 0), stop=False)
                nc.tensor.matmul(kps, bk_sb, ones_b, start=False, stop=True)
                nc.scalar.activation(KT_sb[:, ns], kps, AF.Copy, bias=0.0, scale=1.0)

            # Q^T[64, h, L] = (Wq_h.T @ x^T + bq_h) / 8
            for h in range(HPC):
                hs = slice(HD * h, HD * (h + 1))
                for n in range(NQB):
                    ns = slice(QB * n, QB * (n + 1))
                    qps = psA.tile([HD, QB], F32, tag="st")
                    for f in range(NF):
                        nc.tensor.matmul(qps, wq_sb[:, f, hs], xT_sb[:, f, ns],
                                         start=(f == 0), stop=False)
                    nc.tensor.matmul(qps, bq_sb[:, hs], ones_b, start=False, stop=True)
                    nc.scalar.activation(QT_sb[:, h, ns], qps, AF.Copy,
                                         bias=0.0, scale=SCALE)

            # V[L, 64] = x @ Wv + bv   (natural layout, k on partitions)
            for l in range(NKT):
                ls = slice(KT * l, KT * (l + 1))
                vps = psA.tile([128, HD], F32, tag="st")
                for f in range(NF):
                    nc.tensor.matmul(vps, xT_sb[:, f, ls], wv_sb[:, f, :],
                                     start=(f == 0), stop=False)
                nc.tensor.matmul(vps, ones_b[:, 0:KT], bv_sb, start=False, stop=True)
                nc.vector.tensor_copy(V_sb[:, l, 0:HD], vps)

            # ---- attention ----
            for q in range(NQB):
                qs = slice(QB * q, QB * (q + 1))
                atT = [attp.tile([HD, QB], BF16, tag=f"a{h}", name=f"atT{h}") for h in range(HPC)]
                for h in range(HPC):
                    avps = psB.tile([HD + 1, QB], F32, tag="av")
                    k = 0
                    while k < NKT:
                        gs = min(KG, NKT - k)
                        stps = psA.tile([128, KG, QB], F32, tag="st")
                        ptsb = ptp.tile([128, KG, QB], BF16, tag="pt")
                        for j in range(gs):
                            ks = slice(KT * (k + j), KT * (k + j + 1))
                            nc.tensor.matmul(stps[:, j, :], KT_sb[:, ks],
                                             QT_sb[:, h, qs], start=True, stop=True)
                        nc.scalar.activation(ptsb[:, 0:gs, :], stps[:, 0:gs, :], AF.Exp)
                        for j in range(gs):
                            nc.tensor.matmul(avps, V_sb[:, k + j, :], ptsb[:, j, :],
                                             start=(k + j == 0), stop=(k + j == NKT - 1))
                        k += gs
                    # normalize: rows 0..63 /= row 64
                    rd = nrm.tile([HD + 1, QB], F32, tag="rd")
                    nc.vector.tensor_copy(rd[HD:HD + 1, :], avps[HD:HD + 1, :])
                    nc.vector.reciprocal(rd[HD:HD + 1, :], rd[HD:HD + 1, :])
                    rbps = psC.tile([HD, QB], F32, tag="rb")
                    nc.tensor.matmul(rbps, ones_f[HD:HD + 1, :], rd[HD:HD + 1, :],
                                     start=True, stop=True)
                    rbsb = nrm.tile([HD, QB], F32, tag="rb_sb")
                    nc.vector.tensor_copy(rbsb, rbps)
                    nc.vector.tensor_mul(atT[h], avps[0:HD, :], rbsb)

                # out[Lchunk, 1024] = attnT.T @ Wo  (two C=64 accumulating mms)
                for lc in range(QB // 128):
                    lcs = slice(128 * lc, 128 * (lc + 1))
                    ops = psA.tile([128, 2, QB], F32, tag="st")
                    osb = obp.tile([128, D], F32, tag="ob")
                    for n in range(2):
                        ns = slice(QB * n, QB * (n + 1))
                        nc.tensor.matmul(ops[:, n, :], atT[0][:, lcs], wo0_sb[:, ns],
                                         start=True, stop=False)
                        nc.tensor.matmul(ops[:, n, :], atT[1][:, lcs], wo1_sb[:, ns],
                                         start=False, stop=True)
                    nc.vector.tensor_copy(osb, ops)
                    nc.sync.dma_start(out=out[QB * q + 128 * lc:QB * q + 128 * (lc + 1), :],
                                      in_=osb)
    nc.finalize()
    return nc


def _prep_inputs(x, Wq, bq, Wk, bk, Wv, bv, Wo, bo):
    bf = ml_dtypes.bfloat16
    xT = np.ascontiguousarray(np.asarray(x, dtype=np.float32)[0].T).astype(bf)
    Wq = np.asarray(Wq, dtype=np.float32)
    Wk = np.asarray(Wk, dtype=np.float32)
    Wv = np.asarray(Wv, dtype=np.float32)
    Wo = np.asarray(Wo, dtype=np.float32)
    bq = np.asarray(bq, dtype=np.float32)
    bk = np.asarray(bk, dtype=np.float32)
    bv = np.asarray(bv, dtype=np.float32)
    in_maps = []
    for c in range(NCORES):
        qsl = slice(HPC * HD * c, HPC * HD * (c + 1))   # this core's q-head cols
        kv = c // 2                                     # its kv head
        ksl = slice(HD * kv, HD * (kv + 1))
        in_maps.append({
            "xT": xT,
            "wq": np.ascontiguousarray(Wq[:, qsl]).astype(bf),
            "wk": np.ascontiguousarray(Wk[:, ksl]).astype(bf),
            "wv": np.ascontiguousarray(Wv[:, ksl]).astype(bf),
            "wo0": np.ascontiguousarray(Wo[HPC * HD * c:HPC * HD * c + HD, :]).astype(bf),
            "wo1": np.ascontiguousarray(Wo[HPC * HD * c + HD:HPC * HD * (c + 1), :]).astype(bf),
            "bq": bq[qsl].reshape(1, -1).astype(bf),
            "bk": bk[ksl].reshape(1, -1).astype(bf),
            "bv": bv[ksl].reshape(1, -1).astype(bf),
        })
    return in_maps


def run(inputs, trace=False):
    if "nc" not in _CACHE:
        _CACHE["nc"] = _build()
    nc = _CACHE["nc"]
    in_maps = _prep_inputs(**inputs)
    res = run_bass_kernel_spmd(nc, in_maps, list(range(NCORES)), trace=trace)
    bo = np.asarray(inputs["bo"], dtype=np.float32)
    acc = np.zeros((L, D), dtype=np.float32)
    for r in res.results:
        acc += np.asarray(r["out"], dtype=np.float32)
    out = (acc + bo).reshape(1, L, D)
    return out, res


def kernel(**inputs):
    out, _ = run(inputs, trace=False)
    return out
